# Initial kernel scaffold
#
"""PointNet (segment_reduce) Trainium2 Bass kernel.

Data-parallel over point clouds: 16 graphs x 4096 points sharded as 2
graphs per core across 8 NeuronCores; MLP weights replicated. All
matmuls run in float32r (1 cyc/row). BN is folded into the linear
weights host-side; the identity added to each T-net output is folded
into the final head bias; per-feature bias+ReLU after segment-max uses
relu(max(z)+b) == max(relu(z+b)).
"""
import numpy as np

import concourse.bacc as bacc
import concourse.mybir as mybir
from concourse.tile import TileContext
from concourse.bass_utils import run_bass_kernel_spmd

AF = mybir.ActivationFunctionType
AX = mybir.AxisListType
DT = mybir.dt
ALU = mybir.AluOpType

N_CORES = 8
B = 16                 # graphs
PPG = 4096             # points per graph
GPC = B // N_CORES     # graphs per core
PTS = GPC * PPG        # points per core
CHUNK = 512
NCH = PPG // CHUNK     # chunks per graph (8)
EPS = 1e-5


# ---------------------------------------------------------------- host prep

def _np(x):
    return np.asarray(x, dtype=np.float32)


def _fold(layer):
    """Fold BatchNorm (eval) into (W, b). Returns (W [din,dout], b [dout])."""
    w, b = _np(layer["w"]), _np(layer["b"])
    if "gamma" in layer:
        s = _np(layer["gamma"]) / np.sqrt(_np(layer["var"]) + EPS)
        w = w * s[None, :]
        b = (b - _np(layer["mean"])) * s + _np(layer["beta"])
    return w, b


def _khead(w):
    """[K, M] -> [128, (K//128)*M] with k-tiles side by side along free."""
    K, M = w.shape
    kt = K // 128
    return np.ascontiguousarray(
        w.reshape(kt, 128, M).transpose(1, 0, 2).reshape(128, kt * M))


def _prep(params):
    t = {}

    def stack3(prefix, layers):
        (w1, b1), (w2, b2), (w3, b3) = [_fold(l) for l in layers]
        t[prefix + "W1"] = w1
        t[prefix + "W2"] = w2
        t[prefix + "W3"] = w3
        t[prefix + "b1p"] = np.concatenate([b1, b1]).reshape(128, 1)
        t[prefix + "b2"] = b2.reshape(128, 1)
        # post-max bias, expanded to [128, 16] cols (m*2+g)
        e = np.empty((128, 16), np.float32)
        for m in range(8):
            e[:, 2 * m] = b3[m * 128:(m + 1) * 128]
            e[:, 2 * m + 1] = b3[m * 128:(m + 1) * 128]
        t[prefix + "b3e"] = e

    def head(prefix, layers, eye_d=None):
        (w1, b1), (w2, b2), (w3, b3) = [_fold(l) for l in layers]
        if eye_d is not None:
            b3 = b3 + np.eye(eye_d, dtype=np.float32).reshape(-1)
        t[prefix + "H1"] = _khead(w1)
        t[prefix + "H2"] = _khead(w2)
        t[prefix + "H3"] = _khead(w3)
        t[prefix + "c1"] = b1.reshape(1, -1)
        t[prefix + "c2"] = b2.reshape(1, -1)
        t[prefix + "c3"] = b3.reshape(1, -1)

    stack3("ti_", params["input_tnet"]["in"])
    head("to_", params["input_tnet"]["out"], eye_d=3)
    stack3("tf_", params["feature_tnet"]["in"])
    head("fo_", params["feature_tnet"]["out"], eye_d=64)

    (wa, ba), (wb, bb) = [_fold(l) for l in params["mlp1"]]
    t["m1_Wa"] = wa
    t["m1_Wb"] = wb
    t["m1_bap"] = np.concatenate([ba, ba]).reshape(128, 1)
    t["m1_bbp"] = np.concatenate([bb, bb]).reshape(128, 1)

    stack3("m2_", params["mlp2"])
    head("m3_", params["mlp3"])

    t["ones"] = np.ones((1, 2), np.float32)
    t["ident"] = np.eye(128, dtype=np.float32)
    return t


# f32r-typed inputs (consumed by the PE); everything else float32
_F32R_KEYS = None


def _weight_specs(t):
    specs = {}
    for k, v in t.items():
        f32r = not (k.endswith("b1p") or k.endswith("b2") or k.endswith("b3e")
                    or k.endswith("bap") or k.endswith("bbp") or k == "ident")
        specs[k] = (list(v.shape), f32r)
    return specs


# ---------------------------------------------------------------- device program

def _build(specs):
    nc = bacc.Bacc("TRN2")
    drm = {}
    for k, (shape, f32r) in specs.items():
        drm[k] = nc.dram_tensor(k, shape, DT.float32r if f32r else DT.float32,
                                kind="ExternalInput")
    posT = nc.dram_tensor("posT", [3, PTS], DT.float32r, kind="ExternalInput")
    o_log = nc.dram_tensor("o_log", [GPC, 5], DT.float32, kind="ExternalOutput")
    o_it = nc.dram_tensor("o_it", [GPC, 9], DT.float32, kind="ExternalOutput")
    o_ft = nc.dram_tensor("o_ft", [GPC, 4096], DT.float32, kind="ExternalOutput")

    with TileContext(nc) as tc:
        wpool = tc.tile_pool(name="wpool", bufs=1).__enter__()
        sb = tc.tile_pool(name="sb", bufs=1).__enter__()
        w = {}
        for k, (shape, f32r) in specs.items():
            w[k] = wpool.tile(shape, DT.float32r if f32r else DT.float32,
                              name="w_" + k)
            nc.sync.dma_start(w[k][:, :], drm[k][:, :])
        pos_s = sb.tile([3, PTS], DT.float32r)
        nc.sync.dma_start(pos_s[:, :], posT[:, :])

        # persistent activations
        x1_all = sb.tile([128, PTS // 2], DT.float32r)   # packed mlp1 out
        h2s = sb.tile([128, PPG], DT.float32r, name="h2s")  # one graph's L2 out
        gt = sb.tile([128, 64], DT.float32, name="gt")   # per-group maxes
        gmax = sb.tile([128, 16 * 3], DT.float32, name="gmax")
        hT = sb.tile([128, 16], DT.float32r, name="hT")  # head hidden (ktile,g)
        hT2 = sb.tile([128, 8], DT.float32r, name="hT2")
        hT3 = sb.tile([128, 4], DT.float32r, name="hT3")
        it9 = sb.tile([2, 9], DT.float32r, name="it9")
        ft_s = sb.tile([2, 4096], DT.float32r, name="ft_s")
        log_s = sb.tile([2, 5], DT.float32, name="log_s")
        itL = sb.tile([3, 6], DT.float32r, name="itL")
        ftN = sb.tile([64, 128], DT.float32r, name="ftN")
        WA = sb.tile([3, 128], DT.float32r, name="WA")
        CT = sb.tile([64, 128], DT.float32r, name="CT")
        scr = sb.tile([2, 512], DT.float32, name="scr", bufs=2)  # head staging

        ps = tc.tile_pool(name="ps", bufs=2, space="PSUM").__enter__()

        def sl(g, c):   # chunk slice within a graph's point range
            s = g * PPG + c * CHUNK
            return slice(s, s + CHUNK)

        def phaseA(pass_id, g):
            """layers 1+2 for graph g -> h2s [128, PPG] (f32r)."""
            # L1: 8 chunks packed 2-per-bank into [128, 2048] psum tiles
            for half in range(2):           # 4 chunks' worth per psum tile
                p1 = ps.tile([128, 2048], DT.float32, tag="big", name="p1")
                for q in range(4):
                    c = half * 4 + q
                    lo, hi = (0, 64) if c % 2 == 0 else (64, 128)
                    if pass_id == 0:
                        lhs, rhs = w["ti_W1"], pos_s[:, sl(g, c)]
                    elif pass_id == 1:
                        lhs, rhs = WA[:, 64 * g:64 * g + 64], pos_s[:, sl(g, c)]
                    else:
                        xsrc = x1_all[lo:hi, (g * 4 + c // 2) * CHUNK:
                                      (g * 4 + c // 2 + 1) * CHUNK]
                        lhs, rhs = CT[:, 64 * g:64 * g + 64], xsrc
                    nc.tensor.matmul(p1[lo:hi, (c // 2) * 512:(c // 2 + 1) * 512],
                                     lhs, rhs, start=True, stop=True,
                                     tile_position=(0, lo))
                h1 = sb.tile([128, 2048], DT.float32r, tag="h1", bufs=2, name="h1")
                bias1 = {0: "ti_b1p", 1: "m1_bap", 2: "m2_b1p"}[pass_id]
                nc.scalar.activation(h1[:, :], p1[:, :], AF.Relu,
                                     bias=w[bias1][:, :])
                # L2 (+ for pass 0/2 this is the 64->128 layer; pass 1 has two
                # 64-wide layers then 64->128, handled below)
                if pass_id == 1:
                    # mlp1 L2 (64->64, packed) -> x1_all, then ftnet L1 (64->64)
                    px = ps.tile([128, 2048], DT.float32, tag="big", name="px")
                    for q in range(4):
                        c = half * 4 + q
                        lo, hi = (0, 64) if c % 2 == 0 else (64, 128)
                        nc.tensor.matmul(
                            px[lo:hi, (c // 2) * 512:(c // 2 + 1) * 512],
                            w["m1_Wb"],
                            h1[lo:hi, (c // 2) * 512:(c // 2 + 1) * 512],
                            start=True, stop=True, tile_position=(0, lo))
                    x1sl = x1_all[:, (g * 4 + half * 2) * CHUNK:
                                  (g * 4 + half * 2 + 2) * CHUNK]
                    nc.scalar.activation(x1sl, px[:, :], AF.Relu,
                                         bias=w["m1_bbp"][:, :])
                    pf = ps.tile([128, 2048], DT.float32, tag="big", name="pf")
                    for q in range(4):
                        c = half * 4 + q
                        lo, hi = (0, 64) if c % 2 == 0 else (64, 128)
                        nc.tensor.matmul(
                            pf[lo:hi, (c // 2) * 512:(c // 2 + 1) * 512],
                            w["tf_W1"],
                            x1_all[lo:hi, (g * 4 + half * 2 + c // 2) * CHUNK:
                                   (g * 4 + half * 2 + c // 2 + 1) * CHUNK],
                            start=True, stop=True, tile_position=(0, lo))
                    h1b = sb.tile([128, 2048], DT.float32r, tag="h1b", bufs=2,
                                  name="h1b")
                    nc.scalar.activation(h1b[:, :], pf[:, :], AF.Relu,
                                         bias=w["tf_b1p"][:, :])
                    h1 = h1b
                # 64->128 layer, one chunk at a time (full 128-row output)
                W2 = {0: "ti_W2", 1: "tf_W2", 2: "m2_W2"}[pass_id]
                b2 = {0: "ti_b2", 1: "tf_b2", 2: "m2_b2"}[pass_id]
                p2 = ps.tile([128, 2048], DT.float32, tag="big", name="p2")
                for q in range(4):
                    c = half * 4 + q
                    lo, hi = (0, 64) if c % 2 == 0 else (64, 128)
                    nc.tensor.matmul(p2[:, q * 512:(q + 1) * 512], w[W2],
                                     h1[lo:hi, (c // 2) * 512:(c // 2 + 1) * 512],
                                     start=True, stop=True)
                nc.scalar.activation(h2s[:, half * 2048:(half + 1) * 2048],
                                     p2[:, :], AF.Relu, bias=w[b2][:, :])

        def phaseB(pass_id, g):
            """128->1024 over all chunks + segment max -> gt columns."""
            W3 = {0: "ti_W3", 1: "tf_W3", 2: "m2_W3"}[pass_id]
            for m in range(8):
                for grp in range(2):
                    p3 = ps.tile([128, 2048], DT.float32, tag="big", name="p3")
                    for q in range(4):
                        c = grp * 4 + q
                        nc.tensor.matmul(
                            p3[:, q * 512:(q + 1) * 512],
                            w[W3][:, m * 128:(m + 1) * 128],
                            h2s[:, c * 512:(c + 1) * 512],
                            start=True, stop=True)
                    nc.vector.reduce_max(
                        gt[:, ((m * 2 + g) * 2 + grp):((m * 2 + g) * 2 + grp + 1)],
                        p3[:, :], axis=AX.X)

        def finalize_g(pass_id):
            """gt [128, (m,g,grp)] -> gmax cols; bias+relu -> hT (f32r)."""
            b3e = {0: "ti_b3e", 1: "tf_b3e", 2: "m2_b3e"}[pass_id]
            ga = gmax[:, pass_id * 16:(pass_id + 1) * 16]
            nc.vector.tensor_reduce(
                ga, gt[:, :32].rearrange("p (a b) -> p a b", b=2),
                axis=AX.X, op=ALU.max)
            nc.vector.scalar_tensor_tensor(
                out=ga, in0=ga, scalar=1.0, in1=w[b3e][:, :],
                op0=ALU.mult, op1=ALU.add)
            nc.vector.tensor_scalar_max(hT[:, :], ga, 0.0)

        def head(prefix, mouts, out_tile):
            """Head MLP on hT [128,16]: two relu layers + final linear."""
            cur, ktiles = hT, 8
            for li, (wkey, ckey, mout, nxt) in enumerate(mouts):
                is_last = nxt is None
                for mo in range(0, mout, 512):
                    mw = min(512, mout - mo)
                    po = ps.tile([2, 512], DT.float32, tag="hp", name="po")
                    for k in range(ktiles):
                        nc.tensor.matmul(po[:, :mw], cur[:, 2 * k:2 * k + 2],
                                         w[wkey][:, k * mout + mo:k * mout + mo + mw],
                                         start=(k == 0), stop=False)
                    nc.tensor.matmul(po[:, :mw], w["ones"][:, :],
                                     w[ckey][:, mo:mo + mw],
                                     start=False, stop=True)
                    if is_last:
                        nc.scalar.activation(out_tile[:, mo:mo + mw],
                                             po[:, :mw], AF.Copy)
                    else:
                        s = sb.tile([2, 512], DT.float32, tag="scr2", bufs=2,
                                    name="s")
                        nc.scalar.activation(s[:, :mw], po[:, :mw], AF.Copy)
                        for tt in range(mw // 128):
                            pt = ps.tile([128, 2], DT.float32, tag="hp",
                                         name="pt")
                            nc.tensor.transpose(pt[:, :],
                                                s[:, tt * 128:(tt + 1) * 128],
                                                w["ident"][:2, :2])
                            nc.scalar.activation(
                                nxt[:, (mo // 128 + tt) * 2:(mo // 128 + tt) * 2 + 2],
                                pt[:, :], AF.Relu)
                cur, ktiles = nxt, (mout // 128 if not is_last else ktiles)

        # ---------------- pass 0: input t-net ----------------
        for g in range(GPC):
            phaseA(0, g)
            phaseB(0, g)
        finalize_g(0)
        head("to_", [("to_H1", "to_c1", 512, hT2),
                     ("to_H2", "to_c2", 256, hT3),
                     ("to_H3", "to_c3", 9, None)], it9)
        nc.sync.dma_start(o_it[:, :], it9[:, :].bitcast(DT.float32))
        nc.sync.dma_start(
            itL[:, :].rearrange("i (g j) -> g i j", j=3),
            it9[:, :].rearrange("g (i j) -> g i j", j=3))
        for g in range(GPC):
            pwa = ps.tile([3, 64], DT.float32, tag="hp", name="pwa")
            nc.tensor.matmul(pwa[:, :], itL[:, 3 * g:3 * g + 3],
                             w["m1_Wa"][:, :], start=True, stop=True)
            nc.scalar.activation(WA[:, 64 * g:64 * g + 64], pwa[:, :], AF.Copy)

        # ---------------- pass 1: mlp1 + feature t-net ----------------
        for g in range(GPC):
            phaseA(1, g)
            phaseB(1, g)
        finalize_g(1)
        head("fo_", [("fo_H1", "fo_c1", 512, hT2),
                     ("fo_H2", "fo_c2", 256, hT3),
                     ("fo_H3", "fo_c3", 4096, None)], ft_s)
        nc.sync.dma_start(o_ft[:, :], ft_s[:, :].bitcast(DT.float32))
        nc.sync.dma_start(
            ftN[:, :].rearrange("i (g k) -> g i k", k=64),
            ft_s[:, :].rearrange("g (i k) -> g i k", k=64))
        for g in range(GPC):
            pct = ps.tile([64, 64], DT.float32, tag="hp", name="pct")
            nc.tensor.matmul(pct[:, :], ftN[:, 64 * g:64 * g + 64],
                             w["m2_W1"][:, :], start=True, stop=True)
            nc.scalar.activation(CT[:, 64 * g:64 * g + 64], pct[:, :], AF.Copy)

        # ---------------- pass 2: mlp2 + classifier ----------------
        for g in range(GPC):
            phaseA(2, g)
            phaseB(2, g)
        finalize_g(2)
        head("m3_", [("m3_H1", "m3_c1", 512, hT2),
                     ("m3_H2", "m3_c2", 256, hT3),
                     ("m3_H3", "m3_c3", 5, None)], log_s)
        nc.sync.dma_start(o_log[:, :], log_s[:, :])

        ps.release()
        sb.release()
        wpool.release()
    nc.finalize()
    return nc


# ---------------------------------------------------------------- entry point

_CACHE = {}


def _get_nc(specs_key, specs):
    if specs_key not in _CACHE:
        _CACHE[specs_key] = _build(specs)
    return _CACHE[specs_key]


def _run(pos, batch, params, trace=False):
    t = _prep(params)
    specs = _weight_specs(t)
    nc = _get_nc("v1", specs)

    posT = np.ascontiguousarray(_np(pos).T)          # [3, N]
    in_maps = []
    for c in range(N_CORES):
        m = {k: np.ascontiguousarray(v) for k, v in t.items()}
        m["posT"] = np.ascontiguousarray(posT[:, c * PTS:(c + 1) * PTS])
        in_maps.append(m)
    res = run_bass_kernel_spmd(nc, in_maps, core_ids=list(range(N_CORES)),
                               trace=trace)
    logits = np.concatenate([r["o_log"] for r in res.results], axis=0)
    it = np.concatenate([r["o_it"] for r in res.results], axis=0)
    ft = np.concatenate([r["o_ft"] for r in res.results], axis=0)
    return (logits.astype(np.float32),
            it.reshape(B, 3, 3).astype(np.float32),
            ft.reshape(B, 64, 64).astype(np.float32)), res


def _kernel_numpy(pos, batch, params):
    """Reference fallback for unexpected batch layouts (pure numpy)."""
    pos = _np(pos)
    batch = np.asarray(batch)

    def apply(x, layers):
        for l in layers:
            w, b = _fold(l)
            x = x @ w + b
            if "gamma" in l:
                x = np.maximum(x, 0)
        return x

    def segmax(h):
        out = np.full((B, h.shape[1]), -np.inf, np.float32)
        np.maximum.at(out, batch, h)
        return out

    def tnet(x, tp, d):
        h = apply(x, tp["in"])
        h = segmax(h)
        h = apply(h, tp["out"]).reshape(-1, d, d)
        return np.eye(d, dtype=np.float32)[None] + h

    it = tnet(pos, params["input_tnet"], 3)
    x = np.einsum('nij,nj->ni', it[batch], pos)
    x = apply(x, params["mlp1"])
    ft = tnet(x, params["feature_tnet"], 64)
    x = np.einsum('nij,nj->ni', ft[batch], x)
    x = apply(x, params["mlp2"])
    x = segmax(x)
    logits = apply(x, params["mlp3"])
    return (logits.astype(np.float32), it.astype(np.float32),
            ft.astype(np.float32))


def kernel(pos, batch, params):
    batch = np.asarray(batch)
    expected = np.repeat(np.arange(B, dtype=batch.dtype), PPG)
    if batch.shape != expected.shape or not np.array_equal(batch, expected):
        return _kernel_numpy(pos, batch, params)
    out, _ = _run(pos, batch, params, trace=False)
    return out


# revision 8
# speedup vs baseline: 1.0127x; 1.0127x over previous
"""PointNet (segment_reduce) Trainium2 Bass kernel.

Data-parallel over point clouds: 16 graphs x 4096 points sharded as 2
graphs per core across 8 NeuronCores; MLP weights replicated. All
matmuls run in float32r (1 cyc/row). BatchNorm is folded into the
linear weights host-side; the identity added to each T-net output is
folded into the final head bias; per-feature bias+ReLU after
segment-max uses relu(max(z)+b) == max(relu(z+b)).

Per-core device program (2 graphs, 8192 points, 512-point chunks):
  pass 0: input T-net trunk (3-64-128-1024) + segment max + head -> it
  pass 1: mlp1 (fused with it) + feature T-net trunk + head -> ft
  pass 2: mlp2 (fused with ft) + segment max + classifier head -> logits
Trunk layers run per graph in two phases: A) the small layers into
SBUF, B) the 128->1024 layer into [128,2048] PSUM groups (4 banks,
double buffered) reduced by the vector engine.
"""
import numpy as np

import concourse.bacc as bacc
import concourse.mybir as mybir
from concourse.tile import TileContext
from concourse.bass_utils import run_bass_kernel_spmd

AF = mybir.ActivationFunctionType
AX = mybir.AxisListType
DT = mybir.dt
ALU = mybir.AluOpType

N_CORES = 8
B = 16                 # graphs
PPG = 4096             # points per graph
GPC = B // N_CORES     # graphs per core
PTS = GPC * PPG        # points per core
CHUNK = 512
NCH = PPG // CHUNK     # chunks per graph (8)
EPS = 1e-5

# head weights streamed from DRAM instead of kept resident in SBUF
_STREAMED = {"to_H1", "fo_H1", "m3_H1", "fo_H3", "fo_c3"}


# ---------------------------------------------------------------- host prep

def _np(x):
    return np.asarray(x, dtype=np.float32)


def _fold(layer):
    """Fold BatchNorm (eval mode) into (W, b)."""
    w, b = _np(layer["w"]), _np(layer["b"])
    if "gamma" in layer:
        s = _np(layer["gamma"]) / np.sqrt(_np(layer["var"]) + EPS)
        w = w * s[None, :]
        b = (b - _np(layer["mean"])) * s + _np(layer["beta"])
    return w, b


def _khead(w):
    """[K, M] -> [128, (K//128)*M] with k-tiles side by side."""
    K, M = w.shape
    kt = K // 128
    return np.ascontiguousarray(
        w.reshape(kt, 128, M).transpose(1, 0, 2).reshape(128, kt * M))


def _prep(params):
    t = {}

    def stack3(p, layers):
        (w1, b1), (w2, b2), (w3, b3) = [_fold(l) for l in layers]
        t[p + "W1"], t[p + "W2"], t[p + "W3"] = w1, w2, w3
        t[p + "b1"] = b1.reshape(-1, 1)
        t[p + "b2"] = b2.reshape(-1, 1)
        e = np.empty((128, 16), np.float32)
        for m in range(8):
            e[:, 2 * m] = b3[m * 128:(m + 1) * 128]
            e[:, 2 * m + 1] = b3[m * 128:(m + 1) * 128]
        t[p + "b3e"] = e

    def head(p, layers, eye_d=None):
        (w1, b1), (w2, b2), (w3, b3) = [_fold(l) for l in layers]
        if eye_d is not None:
            b3 = b3 + np.eye(eye_d, dtype=np.float32).reshape(-1)
        if w3.shape[1] % 16:   # fp32r matmul needs even moving free size
            padm = 16 - w3.shape[1] % 16
            w3 = np.pad(w3, ((0, 0), (0, padm)))
            b3 = np.pad(b3, (0, padm))
        t[p + "H1"], t[p + "H2"], t[p + "H3"] = map(_khead, (w1, w2, w3))
        t[p + "c1"] = b1.reshape(1, -1)
        t[p + "c2"] = b2.reshape(1, -1)
        t[p + "c3"] = b3.reshape(1, -1)

    stack3("ti_", params["input_tnet"]["in"])
    head("to_", params["input_tnet"]["out"], eye_d=3)
    stack3("tf_", params["feature_tnet"]["in"])
    head("fo_", params["feature_tnet"]["out"], eye_d=64)

    (wa, ba), (wb, bb) = [_fold(l) for l in params["mlp1"]]
    t["m1_Wa"], t["m1_Wb"] = wa, wb
    t["m1_ba"] = ba.reshape(-1, 1)
    t["m1_bb"] = bb.reshape(-1, 1)

    stack3("m2_", params["mlp2"])
    head("m3_", params["mlp3"])

    t["ones"] = np.ones((1, 2), np.float32)
    t["ident"] = np.eye(128, dtype=np.float32)
    return t


def _weight_specs(t):
    specs = {}
    for k, v in t.items():
        # bias tiles used only by ACT/DVE stay float32; PE inputs are f32r
        plain = (k.endswith("b1") or k.endswith("b2") or k.endswith("b3e")
                 or k.endswith("ba") or k.endswith("bb") or k == "ident")
        specs[k] = (list(v.shape), not plain)
    return specs


# ---------------------------------------------------------------- device program

def _build(specs):
    nc = bacc.Bacc("TRN2")
    drm = {}
    for k, (shape, f32r) in specs.items():
        drm[k] = nc.dram_tensor(k, shape, DT.float32r if f32r else DT.float32,
                                kind="ExternalInput")
    posT_d = nc.dram_tensor("posT", [3, PTS], DT.float32r, kind="ExternalInput")
    o_log = nc.dram_tensor("o_log", [GPC, 5], DT.float32, kind="ExternalOutput")
    o_it = nc.dram_tensor("o_it", [GPC, 9], DT.float32, kind="ExternalOutput")
    o_ft = nc.dram_tensor("o_ft", [GPC, 4096], DT.float32, kind="ExternalOutput")

    from contextlib import ExitStack
    with TileContext(nc) as tc, ExitStack() as ctx:
        wpool = ctx.enter_context(tc.tile_pool(name="wpool", bufs=1))
        sb = ctx.enter_context(tc.tile_pool(name="sb", bufs=1))
        spool = ctx.enter_context(tc.tile_pool(name="spool", bufs=1))
        w = {}
        for k, (shape, f32r) in specs.items():
            if k in _STREAMED:
                continue
            w[k] = wpool.tile(shape, DT.float32r if f32r else DT.float32,
                              name="w_" + k)
            nc.sync.dma_start(w[k][:, :], drm[k][:, :])
        pos_s = sb.tile([3, PTS], DT.float32r)
        nc.sync.dma_start(pos_s[:, :], posT_d[:, :])

        x1_all = sb.tile([64, PTS], DT.float32r)        # mlp1 output
        gt = sb.tile([128, 64], DT.float32, name="gt")
        gmax = sb.tile([128, 48], DT.float32, name="gmax")
        hT = sb.tile([128, 16], DT.float32r, name="hT")
        hT2 = sb.tile([128, 8], DT.float32r, name="hT2")
        hT3 = sb.tile([128, 4], DT.float32r, name="hT3")
        it9 = sb.tile([2, 9], DT.float32r, name="it9")
        ft_s = sb.tile([2, 4096], DT.float32r, name="ft_s")
        log_s = sb.tile([2, 5], DT.float32, name="log_s")
        itL = sb.tile([3, 6], DT.float32r, name="itL")
        ftN = sb.tile([64, 128], DT.float32r, name="ftN")
        WA = sb.tile([3, 128], DT.float32r, name="WA")
        CT = sb.tile([64, 128], DT.float32r, name="CT")

        ps = ctx.enter_context(tc.tile_pool(name="ps", bufs=2, space="PSUM"))

        def sl(g, c):
            s = g * PPG + c * CHUNK
            return slice(s, s + CHUNK)

        def phaseA(pass_id, g, h2s):
            """small trunk layers for graph g -> h2s [128, PPG] f32r."""
            h1 = sb.tile([64, PPG], DT.float32r, tag="h1", bufs=2, name="h1")
            for half in range(2):
                p1 = ps.tile([64, 2048], DT.float32, tag="big", name="p1")
                for q in range(4):
                    c = half * 4 + q
                    if pass_id == 0:
                        lhs, rhs = w["ti_W1"][:, :], pos_s[:, sl(g, c)]
                    elif pass_id == 1:
                        lhs = WA[:, 64 * g:64 * g + 64]
                        rhs = pos_s[:, sl(g, c)]
                    else:
                        lhs = CT[:, 64 * g:64 * g + 64]
                        rhs = x1_all[:, sl(g, c)]
                    nc.tensor.matmul(p1[:, q * 512:(q + 1) * 512], lhs, rhs,
                                     start=True, stop=True)
                bias1 = {0: "ti_b1", 1: "m1_ba", 2: "m2_b1"}[pass_id]
                nc.scalar.activation(h1[:, half * 2048:(half + 1) * 2048],
                                     p1[:, :], AF.Relu, bias=w[bias1][:, :])
            if pass_id == 1:
                # mlp1 L2 (64->64) -> x1_all, then feature-tnet L1 (64->64)
                h1b = sb.tile([64, PPG], DT.float32r, tag="h1", bufs=2,
                              name="h1b")
                for half in range(2):
                    px = ps.tile([64, 2048], DT.float32, tag="big", name="px")
                    for q in range(4):
                        c = half * 4 + q
                        nc.tensor.matmul(px[:, q * 512:(q + 1) * 512],
                                         w["m1_Wb"][:, :],
                                         h1[:, c * 512:(c + 1) * 512],
                                         start=True, stop=True)
                    nc.scalar.activation(
                        x1_all[:, g * PPG + half * 2048:
                               g * PPG + (half + 1) * 2048],
                        px[:, :], AF.Relu, bias=w["m1_bb"][:, :])
                for half in range(2):
                    pf = ps.tile([64, 2048], DT.float32, tag="big", name="pf")
                    for q in range(4):
                        c = half * 4 + q
                        nc.tensor.matmul(pf[:, q * 512:(q + 1) * 512],
                                         w["tf_W1"][:, :],
                                         x1_all[:, sl(g, c)],
                                         start=True, stop=True)
                    nc.scalar.activation(h1b[:, half * 2048:(half + 1) * 2048],
                                         pf[:, :], AF.Relu,
                                         bias=w["tf_b1"][:, :])
                h1 = h1b
            W2 = {0: "ti_W2", 1: "tf_W2", 2: "m2_W2"}[pass_id]
            b2 = {0: "ti_b2", 1: "tf_b2", 2: "m2_b2"}[pass_id]
            for half in range(2):
                p2 = ps.tile([128, 2048], DT.float32, tag="big", name="p2")
                for q in range(4):
                    c = half * 4 + q
                    nc.tensor.matmul(p2[:, q * 512:(q + 1) * 512], w[W2][:, :],
                                     h1[:, c * 512:(c + 1) * 512],
                                     start=True, stop=True)
                nc.scalar.activation(h2s[:, half * 2048:(half + 1) * 2048],
                                     p2[:, :], AF.Relu, bias=w[b2][:, :])

        def phaseB(pass_id, g, h2s):
            """128->1024 over all chunks + per-graph max -> gt columns."""
            W3 = {0: "ti_W3", 1: "tf_W3", 2: "m2_W3"}[pass_id]
            for m in range(8):
                for grp in range(2):
                    p3 = ps.tile([128, 2048], DT.float32, tag="big", name="p3")
                    for q in range(4):
                        c = grp * 4 + q
                        nc.tensor.matmul(p3[:, q * 512:(q + 1) * 512],
                                         w[W3][:, m * 128:(m + 1) * 128],
                                         h2s[:, c * 512:(c + 1) * 512],
                                         start=True, stop=True)
                    col = (m * 2 + g) * 2 + grp
                    nc.vector.reduce_max(gt[:, col:col + 1], p3[:, :],
                                         axis=AX.X)

        def finalize_g(pass_id):
            """gt -> gmax (max over groups), bias+relu -> hT (f32r)."""
            b3e = {0: "ti_b3e", 1: "tf_b3e", 2: "m2_b3e"}[pass_id]
            ga = gmax[:, pass_id * 16:(pass_id + 1) * 16]
            nc.vector.tensor_reduce(
                ga, gt[:, :32].rearrange("p (a b) -> p a b", b=2),
                axis=AX.X, op=ALU.max)
            nc.vector.scalar_tensor_tensor(
                out=ga, in0=ga, scalar=1.0, in1=w[b3e][:, :],
                op0=ALU.mult, op1=ALU.add)
            nc.vector.tensor_scalar_max(hT[:, :], ga, 0.0)

        def head(layers, out_tile):
            """Head MLP: hT [128, 2k] -> ... -> out_tile [2, mout]."""
            cur, ktiles = hT, 8
            for (wkey, ckey, mout, nxt) in layers:
                is_last = nxt is None
                mpad = mout if mout % 16 == 0 else mout + 16 - mout % 16
                for mo in range(0, mpad, 512):
                    mw = min(512, mpad - mo)
                    cw = max(0, min(512, mout - mo))  # true (unpadded) width
                    po = ps.tile([2, 512], DT.float32, tag="big", name="po")
                    for k in range(ktiles):
                        if wkey in _STREAMED:
                            wk = spool.tile([128, 512], DT.float32r,
                                            tag="wstage", bufs=4, name="wk")
                            nc.sync.dma_start(
                                wk[:, :mw],
                                drm[wkey][:, k * mpad + mo:k * mpad + mo + mw])
                            rhs = wk[:, :mw]
                        else:
                            rhs = w[wkey][:, k * mpad + mo:k * mpad + mo + mw]
                        nc.tensor.matmul(po[:, :mw], cur[:, 2 * k:2 * k + 2],
                                         rhs, start=(k == 0), stop=False)
                    if ckey in _STREAMED:
                        ck = spool.tile([1, 512], DT.float32r, tag="cstage",
                                        bufs=2, name="ck")
                        nc.sync.dma_start(ck[:, :mw], drm[ckey][:, mo:mo + mw])
                        crow = ck[:, :mw]
                    else:
                        crow = w[ckey][:, mo:mo + mw]
                    nc.tensor.matmul(po[:, :mw], w["ones"][:, :], crow,
                                     start=False, stop=True)
                    if is_last:
                        nc.scalar.activation(out_tile[:, mo:mo + cw],
                                             po[:, :cw], AF.Copy)
                    else:
                        s = sb.tile([2, 512], DT.float32, tag="scr", bufs=2,
                                    name="s")
                        nc.scalar.activation(s[:, :mw], po[:, :mw], AF.Copy)
                        for tt in range(mw // 128):
                            pt = ps.tile([128, 2], DT.float32, tag="big",
                                         name="pt")
                            nc.tensor.transpose(pt[:, :],
                                                s[:, tt * 128:(tt + 1) * 128],
                                                w["ident"][:2, :2])
                            o = (mo // 128 + tt) * 2
                            nc.scalar.activation(nxt[:, o:o + 2], pt[:, :],
                                                 AF.Relu)
                cur, ktiles = nxt, mout // 128

        def trunk(pass_id):
            for g in range(GPC):
                h2s = sb.tile([128, PPG], DT.float32r, tag="h2s", bufs=2,
                              name="h2s")
                phaseA(pass_id, g, h2s)
                phaseB(pass_id, g, h2s)
            finalize_g(pass_id)

        # ---- pass 0: input t-net ----
        trunk(0)
        head([("to_H1", "to_c1", 512, hT2),
              ("to_H2", "to_c2", 256, hT3),
              ("to_H3", "to_c3", 9, None)], it9)
        nc.sync.dma_start(o_it[:, :], it9[:, :].bitcast(DT.float32))
        nc.sync.dma_start(
            itL[:, :].rearrange("i (g j) -> g i j", j=3),
            it9[:, :].rearrange("g (i j) -> g i j", j=3))
        for g in range(GPC):
            pwa = ps.tile([3, 64], DT.float32, tag="big", name="pwa")
            nc.tensor.matmul(pwa[:, :], itL[:, 3 * g:3 * g + 3],
                             w["m1_Wa"][:, :], start=True, stop=True)
            nc.scalar.activation(WA[:, 64 * g:64 * g + 64], pwa[:, :], AF.Copy)

        # ---- pass 1: mlp1 + feature t-net ----
        trunk(1)
        head([("fo_H1", "fo_c1", 512, hT2),
              ("fo_H2", "fo_c2", 256, hT3),
              ("fo_H3", "fo_c3", 4096, None)], ft_s)
        nc.sync.dma_start(o_ft[:, :], ft_s[:, :].bitcast(DT.float32))
        nc.sync.dma_start(
            ftN[:, :].rearrange("i (g k) -> g i k", k=64),
            ft_s[:, :].rearrange("g (i k) -> g i k", k=64))
        for g in range(GPC):
            pct = ps.tile([64, 64], DT.float32, tag="big", name="pct")
            nc.tensor.matmul(pct[:, :], ftN[:, 64 * g:64 * g + 64],
                             w["m2_W1"][:, :], start=True, stop=True)
            nc.scalar.activation(CT[:, 64 * g:64 * g + 64], pct[:, :], AF.Copy)

        # ---- pass 2: mlp2 + classifier ----
        trunk(2)
        head([("m3_H1", "m3_c1", 512, hT2),
              ("m3_H2", "m3_c2", 256, hT3),
              ("m3_H3", "m3_c3", 5, None)], log_s)
        nc.sync.dma_start(o_log[:, :], log_s[:, :])

    nc.finalize()
    return nc


# ---------------------------------------------------------------- entry point

_CACHE = {}


def _run(pos, batch, params, trace=False):
    t = _prep(params)
    specs = _weight_specs(t)
    if "nc" not in _CACHE:
        _CACHE["nc"] = _build(specs)
    nc = _CACHE["nc"]

    posT = np.ascontiguousarray(_np(pos).T)
    in_maps = []
    for c in range(N_CORES):
        m = {k: np.ascontiguousarray(v) for k, v in t.items()}
        m["posT"] = np.ascontiguousarray(posT[:, c * PTS:(c + 1) * PTS])
        in_maps.append(m)
    res = run_bass_kernel_spmd(nc, in_maps, core_ids=list(range(N_CORES)),
                               trace=trace)
    logits = np.concatenate([r["o_log"] for r in res.results], axis=0)
    it = np.concatenate([r["o_it"] for r in res.results], axis=0)
    ft = np.concatenate([r["o_ft"] for r in res.results], axis=0)
    return (logits.astype(np.float32),
            it.reshape(B, 3, 3).astype(np.float32),
            ft.reshape(B, 64, 64).astype(np.float32)), res


def _kernel_numpy(pos, batch, params):
    """Pure-numpy fallback for unexpected batch layouts."""
    pos = _np(pos)
    batch = np.asarray(batch)

    def apply(x, layers):
        for l in layers:
            w, b = _fold(l)
            x = x @ w + b
            if "gamma" in l:
                x = np.maximum(x, 0)
        return x

    def segmax(h):
        out = np.full((B, h.shape[1]), -np.inf, np.float32)
        np.maximum.at(out, batch, h)
        return out

    def tnet(x, tp, d):
        h = apply(x, tp["in"])
        h = segmax(h)
        h = apply(h, tp["out"]).reshape(-1, d, d)
        return np.eye(d, dtype=np.float32)[None] + h

    it = tnet(pos, params["input_tnet"], 3)
    x = np.einsum('nij,nj->ni', it[batch], pos)
    x = apply(x, params["mlp1"])
    ft = tnet(x, params["feature_tnet"], 64)
    x = np.einsum('nij,nj->ni', ft[batch], x)
    x = apply(x, params["mlp2"])
    x = segmax(x)
    logits = apply(x, params["mlp3"])
    return (logits.astype(np.float32), it.astype(np.float32),
            ft.astype(np.float32))


def kernel(pos, batch, params):
    batch = np.asarray(batch)
    expected = np.repeat(np.arange(B, dtype=batch.dtype), PPG)
    if batch.shape != expected.shape or not np.array_equal(batch, expected):
        return _kernel_numpy(pos, batch, params)
    out, _ = _run(pos, batch, params, trace=False)
    return out
